# revision 19
# baseline (speedup 1.0000x reference)
"""GNN message passing (copy_src + segment_sum + Linear + ReLU) on 8 TRN2 cores.

Strategy: partition dst nodes into 128-node windows; assign windows to the 8
cores (balanced by edge count, serpentine). Each core holds a full bf16
replica of the (split) feature table in its HBM and gathers the src rows of
its edges with dma_gather across 4 SWDGE queues (blocking mode; 4 queues keep
enough DMA transfers in flight that the gather is SDMA-descriptor-service
bound rather than descriptor-gen bound). The int16 gather-index range forces
a two-half table split at 25000.

Edges are packed contiguously per (batch, core): a batch is a group of
windows, and each core lays its batch edges out densely (A-half stream then
B-half stream), so tile-count padding is per batch instead of per window.
Columns fully owned by one window on every core ("interior") use the shared
batch one-hot (DVE is_equal of local dst vs an iota row); columns straddling
windows or varying per core ("fringe") get per-(column, window) one-hot view
columns built from a separate dstloc array whose entries are -1 for edges
not in that window. PE bf16 matmuls accumulate
aggT[f, n] += msgs[e, f].T @ onehot[e, n] into PSUM per window; the node
update (transform with W^T, K=1 bias matmul, ACT ReLU) emits bf16 outputs.
No collectives — the host splits edges and concatenates per-core outputs.

Self-contained: shapes hardcoded for feature[50000,128], src/dst[640000],
W[128,128], b[128].
"""
import dataclasses

import ml_dtypes
import numpy as np

import concourse.bacc as bacc
import concourse.bass as bass
import concourse.tile as tile
from concourse import mybir
from concourse.bass_utils import run_bass_kernel_spmd

P = 128
N_NODES = 50000
N_EDGES = 640000
VHALF = 25000          # feature table split point (int16 gather index range)
NC = 8
G = (N_NODES + P - 1) // P          # 391 global windows
W_SLOTS = (G + NC - 1) // NC        # 49 window slots per core
BATCH_TILES = 64                    # max edge tiles per gather batch
NQ = 4                              # SWDGE queues (ucode max)

F32 = mybir.dt.float32
BF16 = mybir.dt.bfloat16
I16 = mybir.dt.int16
BF = ml_dtypes.bfloat16


def _make_plan(src, dst):
    """Host-side packed partition of edges into (core, batch) streams with
    static, core-uniform tile counts per batch."""
    src = np.asarray(src, dtype=np.int64)
    dst = np.asarray(dst, dtype=np.int64)
    win = dst >> 7
    order = np.lexsort((src, win))          # by window, then src (HBM locality)
    src_s = src[order]
    dst_s = dst[order]
    win_s = win[order]

    cnt = np.bincount(win_s, minlength=G)
    cntA = np.bincount(win_s[src_s < VHALF], minlength=G)
    cntB = cnt - cntA
    start = np.concatenate([[0], np.cumsum(cnt)])[:G]
    cost = cnt

    # serpentine deal of windows (sorted by edge count desc) into cores
    idx_sorted = np.argsort(-cost, kind="stable")
    assign = -np.ones((NC, W_SLOTS), dtype=np.int64)
    for i, g in enumerate(idx_sorted):
        rnd, pos = divmod(i, NC)
        core = pos if (rnd % 2 == 0) else NC - 1 - pos
        assign[core, rnd] = g

    def eAB(c, w):
        g = assign[c, w]
        if g < 0:
            return 0, 0
        return int(cntA[g]), int(cntB[g])

    # greedy batches of slots: packed tile count (max over cores) <= BATCH_TILES
    batches = []
    cur = []
    for w in range(W_SLOTS):
        trial = cur + [w]
        ka = max(-(-sum(eAB(c, x)[0] for x in trial) // P) for c in range(NC))
        kb = max(-(-sum(eAB(c, x)[1] for x in trial) // P) for c in range(NC))
        if cur and ka + kb > BATCH_TILES:
            batches.append(cur)
            cur = [w]
        else:
            cur = trial
    if cur:
        batches.append(cur)
    # final batch = a single window, so the post-gather tail is one window
    if len(batches[-1]) > 1:
        batches.append([batches[-1].pop()])

    # static schedule
    sched = []
    t_base = 0
    a_base = 0
    b_base = 0
    nb_base = 0
    for slots in batches:
        nA = np.array([[eAB(c, w)[0] for w in slots] for c in range(NC)])
        nB = np.array([[eAB(c, w)[1] for w in slots] for c in range(NC)])
        pA = np.concatenate([np.zeros((NC, 1), np.int64),
                             np.cumsum(nA, 1)], 1)
        pB = np.concatenate([np.zeros((NC, 1), np.int64),
                             np.cumsum(nB, 1)], 1)
        ka_b = int(max(-(-int(pA[c, -1]) // P) for c in range(NC)))
        kb_b = int(max(-(-int(pB[c, -1]) // P) for c in range(NC)))
        k_b = ka_b + kb_b

        wins = []
        views = []      # (stream, col, m) ; stream 0=A 1=B; col stream-local
        for m, w in enumerate(slots):
            pairs = []
            holders = [c for c in range(NC) if assign[c, w] >= 0]
            for stream, (ncnt, pfx, koff) in enumerate(
                    [(nA, pA, 0), (nB, pB, ka_b)]):
                act = [c for c in range(NC) if ncnt[c, m] > 0]
                if not act:
                    continue
                lo = min(int(pfx[c, m]) // P for c in act)
                hi = max(-(-int(pfx[c, m + 1]) // P) for c in act)
                # interior: on EVERY holder core, the column holds only
                # window-m edges (or padding)
                ilo = max(-(-int(pfx[c, m]) // P) for c in holders)
                ihi = min(int(pfx[c, m + 1]) // P for c in holders)
                for col in range(lo, hi):
                    if ilo <= col < max(ilo, ihi):
                        pairs.append((koff + col, koff + col))
                    else:
                        views.append((stream, col, m))
                        pairs.append((koff + col, k_b + len(views) - 1))
            if not pairs:
                views.append((0, 0, m))     # all -1 view: bias-only window
                pairs.append((0, k_b + len(views) - 1))
            wins.append((w, pairs))
        sched.append(dict(ka=ka_b, kb=kb_b, nb=len(views), t_base=t_base,
                          a_base=a_base, b_base=b_base, nb_base=nb_base,
                          wins=wins, slots=slots, views=views))
        t_base += k_b
        a_base += ka_b
        b_base += kb_b
        nb_base += len(views)

    TA_tot = sum(bt["ka"] for bt in sched)
    TB_tot = sum(bt["kb"] for bt in sched)
    T_tot = TA_tot + TB_tot
    NB_tot = nb_base

    # per-core data arrays
    dstloc = np.full((NC, P, T_tot), -1.0, dtype=np.float32)
    dstbnd = np.full((NC, P, max(NB_tot, 1)), -1.0, dtype=np.float32)
    idxA = np.zeros((NC, TA_tot * P), dtype=np.int16)
    idxB = np.zeros((NC, TB_tot * P), dtype=np.int16)
    for bt in sched:
        slots = bt["slots"]
        ka_b = bt["ka"]
        for c in range(NC):
            posA = 0
            posB = 0
            spansA = {}
            spansB = {}
            for m, w in enumerate(slots):
                g = assign[c, w]
                if g < 0:
                    continue
                e0 = start[g]
                ca, cb = int(cntA[g]), int(cntB[g])
                dl = (dst_s[e0:e0 + ca + cb] - (g << 7)).astype(np.float32)
                sv = src_s[e0:e0 + ca + cb]
                if ca:
                    j = posA + np.arange(ca)
                    dstloc[c, j % P, bt["t_base"] + j // P] = dl[:ca]
                    idxA[c, bt["a_base"] * P + j] = sv[:ca].astype(np.int16)
                    spansA[m] = (posA, posA + ca, dl[:ca])
                    posA += ca
                if cb:
                    j = posB + np.arange(cb)
                    dstloc[c, j % P, bt["t_base"] + ka_b + j // P] = dl[ca:]
                    idxB[c, bt["b_base"] * P + j] = \
                        (sv[ca:] - VHALF).astype(np.int16)
                    spansB[m] = (posB, posB + cb, dl[ca:])
                    posB += cb
            for vi, (stream, col, m) in enumerate(bt["views"]):
                spans = spansA if stream == 0 else spansB
                if m not in spans:
                    continue
                s0, s1, dl = spans[m]
                j0 = max(s0, col * P)
                j1 = min(s1, (col + 1) * P)
                if j0 >= j1:
                    continue
                j = np.arange(j0, j1)
                dstbnd[c, j % P, bt["nb_base"] + vi] = dl[j - s0]

    return dict(sched=sched, assign=assign, cnt=cnt,
                TA_tot=TA_tot, TB_tot=TB_tot, T_tot=T_tot, NB_tot=NB_tot,
                dstloc=dstloc, dstbnd=dstbnd, idxA=idxA, idxB=idxB)


def _wrap16(idx_flat):
    """Logical int16 index list [n*P] -> [128, n*8] (16-wrap, replicated 8x)."""
    n = idx_flat.shape[0]
    assert n % 16 == 0
    arr = np.empty((16, n // 16), dtype=np.int16)
    j = np.arange(n)
    arr[j % 16, j // 16] = idx_flat
    return np.tile(arr, (8, 1))


def _build_nc(plan):
    T_tot, TA_tot, TB_tot = plan["T_tot"], plan["TA_tot"], plan["TB_tot"]
    NB_tot = max(plan["NB_tot"], 1)
    # const int16-column layout: dstloc(f32) | dstbnd(f32) | idxA | idxB |
    # iota(f32) | wt(bf16) | brow(bf16)
    c_dst = 0
    c_bnd = T_tot * 2
    c_ia = c_bnd + NB_tot * 2
    c_ib = c_ia + TA_tot * 8
    c_iota = c_ib + TB_tot * 8
    c_wt = c_iota + P * 2
    c_brow = c_wt + P
    c_tot = c_brow + P
    plan["c_layout"] = (c_dst, c_bnd, c_ia, c_ib, c_iota, c_wt, c_brow, c_tot)

    nc = bacc.Bacc("TRN2", num_swdge_queues=NQ)
    featA = nc.declare_dram_parameter("featA", [VHALF, P], BF16, isOutput=False)
    featB = nc.declare_dram_parameter("featB", [N_NODES - VHALF, P], BF16,
                                      isOutput=False)
    consts = nc.declare_dram_parameter("consts", [P, c_tot], I16, isOutput=False)
    out = nc.declare_dram_parameter("out", [W_SLOTS * P, P], BF16, isOutput=True)

    with tile.TileContext(nc) as tc:
        with (
            tc.tile_pool(name="const", bufs=1) as const_pool,
            tc.tile_pool(name="work", bufs=3) as work_pool,
            tc.tile_pool(name="outp", bufs=2) as out_pool,
            tc.tile_pool(name="psum", bufs=4, space="PSUM") as psum_pool,
        ):
            cs = const_pool.tile([P, c_tot], I16)
            ka0 = plan["sched"][0]["ka"]
            kb0 = plan["sched"][0]["kb"]
            b0ia = const_pool.tile([P, ka0 * 8], I16)
            nc.sync.dma_start(out=b0ia[:], in_=consts[:, c_ia:c_ia + ka0 * 8])
            b0ib = const_pool.tile([P, kb0 * 8], I16)
            nc.sync.dma_start(out=b0ib[:], in_=consts[:, c_ib:c_ib + kb0 * 8])
            nc.scalar.dma_start(out=cs[:], in_=consts[:])
            idxA_sb = cs[:, c_ia:c_ia + TA_tot * 8]
            idxB_sb = cs[:, c_ib:c_ib + TB_tot * 8]
            dst_sb = cs[:, c_dst:c_dst + T_tot * 2].bitcast(F32)
            bnd_sb = cs[:, c_bnd:c_bnd + NB_tot * 2].bitcast(F32)
            iota_sb = cs[:, c_iota:c_iota + P * 2].bitcast(F32)
            wt_sb = cs[:, c_wt:c_wt + P].bitcast(BF16)
            brow_sb = cs[0:1, c_brow:c_brow + P].bitcast(BF16)
            ones_sb = const_pool.tile([1, P], BF16)
            nc.vector.memset(ones_sb[:], 1.0)

            def iota_bcast(k):
                return dataclasses.replace(
                    iota_sb, ap=[iota_sb.ap[0], [0, k], iota_sb.ap[1]])

            gq = [0]
            for bi, bt in enumerate(plan["sched"]):
                ka_b, kb_b, nb_b = bt["ka"], bt["kb"], bt["nb"]
                k_b = ka_b + kb_b
                if bi == 0:
                    ia_t = b0ia[:]
                    ib_t = b0ib[:]
                else:
                    ia_t = idxA_sb[:, bt["a_base"] * 8:
                                   (bt["a_base"] + ka_b) * 8]
                    ib_t = idxB_sb[:, bt["b_base"] * 8:
                                   (bt["b_base"] + kb_b) * 8]
                msgs = work_pool.tile([P, k_b, P], BF16, tag="msgs")
                # HW limit: <=1024 indices (8 tiles) per dma_gather (SWDGE
                # descriptor-ring capacity of 64 descs/engine). The final
                # batch uses small chunks so the end-of-kernel DMA pipeline
                # drains quickly.
                ck = 2 if bi == len(plan["sched"]) - 1 else 8
                def _chunks(total):
                    return [(o, min(ck, total - o)) for o in range(0, total, ck)]

                for off, nk in _chunks(ka_b):
                    nc.gpsimd.dma_gather(
                        out_ap=msgs[:, off:off + nk, :],
                        in_ap=featA[:],
                        idxs_ap=ia_t[:, off * 8:(off + nk) * 8],
                        num_idxs=nk * P,
                        num_idxs_reg=nk * P,
                        elem_size=P,
                        queue_num=gq[0] % NQ,
                    )
                    gq[0] += 1
                for off, nk in _chunks(kb_b):
                    nc.gpsimd.dma_gather(
                        out_ap=msgs[:, ka_b + off:ka_b + off + nk, :],
                        in_ap=featB[:],
                        idxs_ap=ib_t[:, off * 8:(off + nk) * 8],
                        num_idxs=nk * P,
                        num_idxs_reg=nk * P,
                        elem_size=P,
                        queue_num=gq[0] % NQ,
                    )
                    gq[0] += 1
                onehot = work_pool.tile([P, k_b + nb_b, P], BF16, tag="onehot")
                nc.vector.tensor_tensor(
                    out=onehot[:, 0:k_b, :],
                    in0=dst_sb[:, bt["t_base"]:bt["t_base"] + k_b]
                        .to_broadcast([P, k_b, P]),
                    in1=iota_bcast(k_b),
                    op=mybir.AluOpType.is_equal,
                )
                if nb_b:
                    nc.vector.tensor_tensor(
                        out=onehot[:, k_b:k_b + nb_b, :],
                        in0=bnd_sb[:, bt["nb_base"]:bt["nb_base"] + nb_b]
                            .to_broadcast([P, nb_b, P]),
                        in1=iota_bcast(nb_b),
                        op=mybir.AluOpType.is_equal,
                    )
                for w, pairs in bt["wins"]:
                    aggT_ps = psum_pool.tile([P, P], F32, tag="aggT")
                    for i, (mc, oc) in enumerate(pairs):
                        nc.tensor.matmul(
                            out=aggT_ps[:],
                            lhsT=msgs[:, mc, :],
                            rhs=onehot[:, oc, :],
                            start=(i == 0),
                            stop=(i == len(pairs) - 1),
                        )
                    aggT_sb = out_pool.tile([P, P], BF16, tag="aggT_sb")
                    nc.vector.tensor_copy(out=aggT_sb[:], in_=aggT_ps[:])
                    out2_ps = psum_pool.tile([P, P], F32, tag="out2")
                    nc.tensor.matmul(out=out2_ps[:], lhsT=aggT_sb[:], rhs=wt_sb,
                                     start=True, stop=False)
                    nc.tensor.matmul(out=out2_ps[:], lhsT=ones_sb[:], rhs=brow_sb,
                                     start=False, stop=True)
                    out_sb = out_pool.tile([P, P], BF16, tag="out_sb")
                    nc.scalar.activation(out=out_sb[:], in_=out2_ps[:],
                                         func=mybir.ActivationFunctionType.Relu)
                    nc.sync.dma_start(out=out[w * P:(w + 1) * P, :],
                                      in_=out_sb[:])
    nc.finalize()
    return nc


_CACHE = {}


def _prepare(feature, src, dst, W, b):
    feature = np.asarray(feature, dtype=np.float32)
    W = np.asarray(W, dtype=np.float32)
    b = np.asarray(b, dtype=np.float32)
    key = (hash(np.asarray(src).tobytes()), hash(np.asarray(dst).tobytes()))
    if key not in _CACHE:
        plan = _make_plan(src, dst)
        nc = _build_nc(plan)
        _CACHE.clear()
        _CACHE[key] = (plan, nc)
    plan, nc = _CACHE[key]
    c_dst, c_bnd, c_ia, c_ib, c_iota, c_wt, c_brow, c_tot = plan["c_layout"]
    NB_tot = max(plan["NB_tot"], 1)
    iota16 = np.arange(P, dtype=np.float32).view(np.int16)
    in_maps = []
    featA = np.ascontiguousarray(feature[:VHALF].astype(BF))
    featB = np.ascontiguousarray(feature[VHALF:].astype(BF))
    wt16 = np.ascontiguousarray(W.T.astype(BF)).view(np.int16)
    b16 = b.astype(BF).view(np.int16)
    for c in range(NC):
        consts = np.zeros((P, c_tot), dtype=np.int16)
        consts[:, c_dst:c_dst + plan["T_tot"] * 2] = \
            plan["dstloc"][c].view(np.int16)
        consts[:, c_bnd:c_bnd + NB_tot * 2] = plan["dstbnd"][c].view(np.int16)
        if plan["TA_tot"]:
            consts[:, c_ia:c_ia + plan["TA_tot"] * 8] = _wrap16(plan["idxA"][c])
        if plan["TB_tot"]:
            consts[:, c_ib:c_ib + plan["TB_tot"] * 8] = _wrap16(plan["idxB"][c])
        consts[:, c_iota:c_iota + P * 2] = iota16[None, :]
        consts[:, c_wt:c_wt + P] = wt16
        consts[0, c_brow:c_brow + P] = b16
        in_maps.append({"featA": featA, "featB": featB, "consts": consts})
    return plan, nc, in_maps


def _assemble(plan, results):
    out_full = np.zeros((N_NODES, P), dtype=np.float32)
    assign = plan["assign"]
    for c in range(NC):
        oc = np.asarray(results[c]["out"]).astype(np.float32)
        for w in range(W_SLOTS):
            g = assign[c, w]
            if g < 0:
                continue
            n0 = int(g) << 7
            n1 = min(n0 + P, N_NODES)
            out_full[n0:n1] = oc[w * P:w * P + (n1 - n0)]
    return out_full


def kernel(feature, src, dst, W, b):
    plan, nc, in_maps = _prepare(feature, src, dst, W, b)
    res = run_bass_kernel_spmd(nc, in_maps, list(range(NC)))
    return _assemble(plan, res.results)


def kernel_traced(feature, src, dst, W, b, **trace_kwargs):
    """Like kernel() but returns (output, BassKernelResults) with trace."""
    plan, nc, in_maps = _prepare(feature, src, dst, W, b)
    res = run_bass_kernel_spmd(nc, in_maps, list(range(NC)), trace=True,
                               **trace_kwargs)
    return _assemble(plan, res.results), res
